# revision 2
# baseline (speedup 1.0000x reference)
"""Trainium2 Bass kernel for nn_DecoderRoPEBlock (B=4, LQ=1024, LC=512,
E=1024, H=16, FF=4096) running SPMD on 8 NeuronCores.

Sharding: 8 cores = (batch, query-half); zero collectives. Stage-1 causal
self-attention K/V are recomputed per core from the original x (causality
means the pre-residual x suffices), so each core produces its 512 output
rows completely independently. See the builder docstring below for the
on-chip design.
"""
import sys
sys.path.insert(0, '/opt/trn_rl_repo')
from contextlib import ExitStack

import numpy as np
import ml_dtypes

import concourse.bass as bass
import concourse.tile as tile
import concourse.mybir as mybir

f32 = mybir.dt.float32
bf16 = mybir.dt.bfloat16
AF = mybir.ActivationFunctionType
ALU = mybir.AluOpType
EPS = 1e-6
P = 128


class Cfg:
    def __init__(self, E, H, LQ, LC, B, FF, n_cores):
        self.E, self.H, self.LQ, self.LC, self.B, self.FF = E, H, LQ, LC, B, FF
        self.HD = E // H
        assert self.HD == 64, "rope layout assumes head dim 64"
        self.n_cores = n_cores
        self.qsplit = n_cores // B
        assert B * self.qsplit == n_cores
        self.Lq = LQ // self.qsplit
        assert self.Lq <= 512
        self.Lk = LQ
        self.Lc = LC
        self.nec = E // P
        self.nkt = self.Lk // P
        self.nct = self.Lc // P
        self.npr = H // 2
        assert self.npr == self.nec, "2 heads per 128-chunk layout"
        self.nft = FF // P
        self.NT = 512


def _swap32_dma(nc, dst, src, L):
    """dst = src with 32-blocks swapped inside each 64-block (partitions)."""
    ps_d = dst.ap[0][0]
    ps_s = src.ap[0][0]

    def view(ap, ps, poff):
        return bass.AP(tensor=ap.tensor, offset=ap.offset + poff * ps,
                       ap=[[64 * ps, 2], [ps, 32], [1, L]])
    nc.sync.dma_start(out=view(dst, ps_d, 0), in_=view(src, ps_s, 32))
    nc.sync.dma_start(out=view(dst, ps_d, 32), in_=view(src, ps_s, 0))


def build_core_program(cfg: Cfg):
    c = cfg
    nc = bass.Bass()

    d_xT = nc.declare_dram_parameter("xT", [c.E, c.Lq], f32, isOutput=False)
    d_xT16 = nc.declare_dram_parameter("xT16", [c.E, c.Lk], bf16, isOutput=False)
    d_ctxT16 = nc.declare_dram_parameter("ctxT16", [c.E, c.Lc], bf16, isOutput=False)
    d_mask = nc.declare_dram_parameter("mask16", [c.Lk, c.Lq], bf16, isOutput=False)
    WNAMES = ["sa_q", "sa_k", "sa_v", "sa_p", "ca_q", "ca_k", "ca_v", "ca_p"]
    d_w = {n: nc.declare_dram_parameter("w_" + n, [c.E, c.E], bf16, isOutput=False)
           for n in WNAMES}
    d_fc1 = nc.declare_dram_parameter("w_fc1", [c.E, c.FF], bf16, isOutput=False)
    d_fc2 = nc.declare_dram_parameter("w_fc2", [c.FF, c.E], bf16, isOutput=False)
    d_cq = nc.declare_dram_parameter("cos_q", [P, c.Lq], bf16, isOutput=False)
    d_sq = nc.declare_dram_parameter("sin_q", [P, c.Lq], bf16, isOutput=False)
    d_ck = nc.declare_dram_parameter("cos_k", [P, c.Lk], bf16, isOutput=False)
    d_sk = nc.declare_dram_parameter("sin_k", [P, c.Lk], bf16, isOutput=False)
    d_cc = nc.declare_dram_parameter("cos_c", [P, c.Lc], bf16, isOutput=False)
    d_sc = nc.declare_dram_parameter("sin_c", [P, c.Lc], bf16, isOutput=False)
    d_g = nc.declare_dram_parameter("gvec", [c.E, 3], f32, isOutput=False)
    d_out = nc.declare_dram_parameter("outT", [c.E, c.Lq], f32, isOutput=True)

    Lq, Lk, Lc, nec, nkt, nct, npr, nft = (
        c.Lq, c.Lk, c.Lc, c.nec, c.nkt, c.nct, c.npr, c.nft)
    VNT = min(c.NT, c.E)
    n_vnt = c.E // VNT
    KNT = min(c.NT, Lk)
    n_knt = Lk // KNT
    NG = 4 if nft % 4 == 0 else 1          # fc1/fc2 stream groups
    FG = c.FF // NG                        # cols per fc1 group
    nftg = nft // NG

    with ExitStack() as ctx, tile.TileContext(nc) as tc:
        # -------------------- pools --------------------
        p_x = ctx.enter_context(tc.tile_pool(name="p_x", bufs=1))
        p_h = ctx.enter_context(tc.tile_pool(name="p_h", bufs=1))
        p_big = ctx.enter_context(tc.tile_pool(name="p_big", bufs=9))
        p_s1k = ctx.enter_context(tc.tile_pool(name="p_s1k", bufs=34))
        p_w = ctx.enter_context(tc.tile_pool(name="p_w", bufs=10))
        p_per = ctx.enter_context(tc.tile_pool(name="p_per", bufs=1))
        p_scr = ctx.enter_context(tc.tile_pool(name="p_scr", bufs=1))
        p_sm = ctx.enter_context(tc.tile_pool(name="p_sm", bufs=4))
        p_t = ctx.enter_context(tc.tile_pool(name="p_t", bufs=4))
        ps_mm = ctx.enter_context(tc.tile_pool(name="ps_mm", bufs=2, space="PSUM"))
        ps_av = ctx.enter_context(tc.tile_pool(name="ps_av", bufs=2, space="PSUM"))
        ps_st = ctx.enter_context(tc.tile_pool(name="ps_st", bufs=1, space="PSUM"))
        ps_bc = ctx.enter_context(tc.tile_pool(name="ps_bc", bufs=2, space="PSUM"))

        # -------------------- prologue loads --------------------
        xT = []
        for e in range(nec):
            t = p_x.tile([P, Lq], f32, tag=f"x{e}")
            nc.sync.dma_start(out=t[:], in_=d_xT[e * P:(e + 1) * P, :])
            xT.append(t)
        x16 = []
        for e in range(nec):
            t = p_h.tile([P, Lk], bf16, tag=f"h{e}")
            nc.sync.dma_start(out=t[:], in_=d_xT16[e * P:(e + 1) * P, :])
            x16.append(t)
        masks = []
        for kt in range(nkt):
            t = p_per.tile([P, Lq], bf16, tag=f"mask{kt}")
            nc.sync.dma_start(out=t[:], in_=d_mask[kt * P:(kt + 1) * P, :])
            masks.append(t)
        cq = p_per.tile([P, Lq], bf16, tag="cq")
        sq = p_per.tile([P, Lq], bf16, tag="sq")
        ck = p_per.tile([P, Lk], bf16, tag="ck")
        sk = p_per.tile([P, Lk], bf16, tag="sk")
        ccos = p_per.tile([P, Lc], bf16, tag="ccos")
        csin = p_per.tile([P, Lc], bf16, tag="csin")
        for t, d in ((cq, d_cq), (sq, d_sq), (ck, d_ck), (sk, d_sk),
                     (ccos, d_cc), (csin, d_sc)):
            nc.sync.dma_start(out=t[:], in_=d[:, :])
        gsb = p_per.tile([P, nec, 3], f32, tag="g")
        for e in range(nec):
            nc.sync.dma_start(out=gsb[:, e, :], in_=d_g[e * P:(e + 1) * P, :])
        ones_col = p_per.tile([P, 1], bf16, tag="ones_col")
        nc.vector.memset(ones_col[:], 1.0)
        ones_row = p_per.tile([1, P], bf16, tag="ones_row")
        nc.vector.memset(ones_row[:], 1.0)
        epsb = p_per.tile([1, 1], f32, tag="epsb")
        nc.vector.memset(epsb[:], EPS)

        def load_w(dram, tag="wproj"):
            tiles = []
            for e in range(nec):
                t = p_w.tile([P, c.E], bf16, tag=tag)
                nc.sync.dma_start(out=t[:], in_=dram[e * P:(e + 1) * P, :])
                tiles.append(t)
            return tiles

        # ==================== LN ====================
        def layer_norm(src_tiles, L, sq_tag, src_f32=None):
            """LN over E of transposed src [E, L] -> bf16 h tiles (p_h h{e})."""
            n_lt = max(1, L // 512)
            LT = L // n_lt
            sq_t = []
            for e in range(nec):
                s = p_scr.tile([P, L], bf16, tag=f"{sq_tag}{e}")
                nc.vector.tensor_mul(s[:], src_tiles[e][:], src_tiles[e][:])
                sq_t.append(s)
            hs = [p_h.tile([P, L], bf16, tag=f"h{e}") for e in range(nec)]
            for lt in range(n_lt):
                sl = slice(lt * LT, (lt + 1) * LT)
                s1 = ps_st.tile([1, LT], f32, tag="s1")
                s2 = ps_st.tile([1, LT], f32, tag="s2")
                for e in range(nec):
                    nc.tensor.matmul(s1[:], ones_col[:], src_tiles[e][:, sl],
                                     start=(e == 0), stop=(e == nec - 1))
                for e in range(nec):
                    nc.tensor.matmul(s2[:], ones_col[:], sq_t[e][:, sl],
                                     start=(e == 0), stop=(e == nec - 1))
                mu = p_sm.tile([1, LT], f32, tag="mu")
                nc.scalar.mul(mu[:], s1[:], 1.0 / c.E)
                mu2 = p_sm.tile([1, LT], f32, tag="mu2")
                nc.scalar.square(mu2[:], mu[:])
                var = p_sm.tile([1, LT], f32, tag="var")
                nc.vector.scalar_tensor_tensor(
                    out=var[:], in0=s2[:], scalar=1.0 / c.E, in1=mu2[:],
                    op0=ALU.mult, op1=ALU.subtract)
                lnv = p_sm.tile([1, LT], f32, tag="lnv")
                nc.scalar.activation(out=lnv[:], in_=var[:], func=AF.Ln,
                                     bias=epsb[:])
                rstd = p_sm.tile([1, LT], bf16, tag="rstd")
                nc.scalar.activation(out=rstd[:], in_=lnv[:], func=AF.Exp,
                                     scale=-0.5)
                ccv = p_sm.tile([1, LT], bf16, tag="ccv")
                nc.vector.tensor_mul(ccv[:], mu[:], rstd[:])
                rstd_b = ps_bc.tile([P, LT], f32, tag="bc")
                nc.tensor.matmul(rstd_b[:], ones_row[:], rstd[:],
                                 start=True, stop=True)
                cc_b = ps_bc.tile([P, LT], f32, tag="bc")
                nc.tensor.matmul(cc_b[:], ones_row[:], ccv[:],
                                 start=True, stop=True)
                for e in range(nec):
                    src = src_f32[e] if src_f32 is not None else src_tiles[e]
                    tmp = p_t.tile([P, LT], f32, tag="lntmp")
                    nc.vector.tensor_mul(tmp[:], src[:, sl], rstd_b[:])
                    nc.vector.tensor_sub(hs[e][:, sl], tmp[:], cc_b[:])
            return hs

        # ==================== projections ====================
        def project_T(w_tiles, rhs_tiles, L, out_tag, pool):
            """[E_out, L] = sum_e w[e].T @ rhs[e]; returns nec bf16 tiles."""
            outs = []
            for eo in range(nec):
                ps = ps_mm.tile([P, L], f32, tag="mm")
                for ei in range(nec):
                    nc.tensor.matmul(ps[:], w_tiles[ei][:, eo * P:(eo + 1) * P],
                                     rhs_tiles[ei][:], start=(ei == 0),
                                     stop=(ei == nec - 1))
                o = pool.tile([P, L], bf16, tag=f"{out_tag}{eo}")
                nc.scalar.copy(o[:], ps[:])
                outs.append(o)
            return outs

        def rope_combine(dst_ap, raw_tile, cos_ap, sin_ap, L):
            swp = p_t.tile([P, L], bf16, tag="ropeswp")
            _swap32_dma(nc, swp[:], raw_tile[:], L)
            t1 = p_t.tile([P, L], bf16, tag="ropet1")
            nc.vector.tensor_mul(t1[:], raw_tile[:], cos_ap)
            t2 = p_t.tile([P, L], bf16, tag="ropet2")
            nc.vector.tensor_mul(t2[:], swp[:], sin_ap)
            nc.vector.tensor_add(dst_ap, t1[:], t2[:])

        def v_project(w_tiles, rhs_tiles, n_kt, rhs_off):
            """V in token layout with per-head ones column: [k, H*65]."""
            v_sb = []
            for kt in range(n_kt):
                vt = p_per.tile([P, c.H, 65], bf16, tag=f"v{kt}")
                nc.vector.memset(vt[:, :, 64:65], 1.0)
                for vn in range(n_vnt):
                    ps = ps_mm.tile([P, VNT], f32, tag="mm")
                    for ei in range(nec):
                        nc.tensor.matmul(
                            ps[:],
                            rhs_tiles[ei][:, rhs_off + kt * P:rhs_off + (kt + 1) * P],
                            w_tiles[ei][:, vn * VNT:(vn + 1) * VNT],
                            start=(ei == 0), stop=(ei == nec - 1))
                    nh = VNT // 64
                    nc.scalar.copy(
                        vt[:, vn * nh:(vn + 1) * nh, 0:64],
                        ps[:].rearrange("p (nh d) -> p nh d", d=64))
                v_sb.append(vt)
            return v_sb

        # ==================== attention ====================
        def attention(q_tiles, k_tiles, v_sb, n_kt, use_mask):
            """Returns Onorm tiles (aliasing q_tiles' slots, tag qt{pr})."""
            on_tiles = []
            for pr in range(npr):
                qch = q_tiles[pr]
                kch = k_tiles[pr]
                results = []  # (pbase, o_ps, db)
                for hh, pbase in ((2 * pr, 0), (2 * pr + 1, 64)):
                    o_ps = ps_av.tile([65, Lq], f32, tag="av")
                    exps = []
                    for kt in range(n_kt):
                        s_ps = ps_mm.tile([P, Lq], f32, tag="mm")
                        nc.tensor.matmul(
                            s_ps[:],
                            kch[pbase:pbase + 64, kt * P:(kt + 1) * P],
                            qch[pbase:pbase + 64, :],
                            start=True, stop=True)
                        ex = p_s1k.tile([P, Lq], bf16, tag="s1k")
                        nc.scalar.activation(out=ex[:], in_=s_ps[:],
                                             func=AF.Exp, scale=0.125)
                        if use_mask:
                            exm = p_s1k.tile([P, Lq], bf16, tag="s1k")
                            nc.vector.tensor_mul(exm[:], ex[:], masks[kt][:])
                            ex = exm
                        exps.append(ex)
                    for kt in range(n_kt):
                        nc.tensor.matmul(o_ps[:],
                                         v_sb[kt][:, hh, :],
                                         exps[kt][:],
                                         start=(kt == 0), stop=(kt == n_kt - 1))
                    rec = p_sm.tile([1, Lq], bf16, tag="rec")
                    nc.vector.reciprocal(rec[:], o_ps[64:65, :])
                    db_ps = ps_bc.tile([P, Lq], f32, tag="bc")
                    nc.tensor.matmul(db_ps[0:64, :], ones_row[:, 0:64], rec[:],
                                     start=True, stop=True)
                    db = p_t.tile([64, Lq], bf16, tag="db")
                    nc.scalar.copy(db[:], db_ps[0:64, :])
                    results.append((pbase, o_ps, db))
                # Onorm tile reuses the pair's q slot (q dead after scores)
                on = p_per.tile([P, Lq], bf16, tag=f"qt{pr}")
                for pbase, o_ps, db in results:
                    nc.vector.scalar_tensor_tensor(
                        out=on[pbase:pbase + 64, :],
                        in0=o_ps[0:64, :], scalar=1.0, in1=db[:],
                        op0=ALU.bypass, op1=ALU.mult)
                on_tiles.append(on)
            return on_tiles

        def proj_residual(w_tiles, src_tiles, g_idx):
            for e in range(nec):
                ps = ps_mm.tile([P, Lq], f32, tag="mm")
                for ei in range(nec):
                    nc.tensor.matmul(ps[:], w_tiles[ei][:, e * P:(e + 1) * P],
                                     src_tiles[ei][:], start=(ei == 0),
                                     stop=(ei == nec - 1))
                nc.vector.scalar_tensor_tensor(
                    out=xT[e][:], in0=ps[:], scalar=gsb[:, e, g_idx:g_idx + 1],
                    in1=xT[e][:], op0=ALU.mult, op1=ALU.add)

        # ==================== STAGE 1: causal self-attention ============
        h1 = layer_norm(x16, Lk, "scrA")
        hq = [t[:, 0:Lq] for t in h1]

        w = load_w(d_w["sa_q"])
        q_raw = project_T(w, hq, Lq, "scrC", p_scr)
        qt1 = []
        for pr in range(npr):
            q = p_per.tile([P, Lq], bf16, tag=f"qt{pr}")
            rope_combine(q[:], q_raw[pr], cq[:], sq[:], Lq)
            qt1.append(q)
        w = load_w(d_w["sa_k"])
        kt1 = []
        k_raw_nt = []
        for nt in range(n_knt):
            sl = slice(nt * KNT, (nt + 1) * KNT)
            tag = "scrA" if nt == 0 else "scrB"
            kr = project_T(w, [t[:, sl] for t in h1], KNT, tag, p_scr)
            k_raw_nt.append(kr)
        for e in range(nec):
            full = p_big.tile([P, Lk], bf16, tag="big")
            for nt in range(n_knt):
                sl = slice(nt * KNT, (nt + 1) * KNT)
                rope_combine(full[:, sl], k_raw_nt[nt][e],
                             ck[:, sl], sk[:, sl], KNT)
            kt1.append(full)
        w = load_w(d_w["sa_v"])
        v1 = v_project(w, h1, nkt, 0)
        on1 = attention(qt1, kt1, v1, nkt, True)
        w = load_w(d_w["sa_p"])
        proj_residual(w, on1, 0)

        # ==================== STAGE 2: cross-attention ==================
        x16_2 = []
        for e in range(nec):
            t = p_scr.tile([P, Lq], bf16, tag=f"scrB{e}")
            nc.scalar.copy(t[:], xT[e][:])
            x16_2.append(t)
        h2 = layer_norm(x16_2, Lq, "scrA", src_f32=xT)
        ctx16 = []
        for e in range(nec):
            tag = f"v{4 + e}" if 4 + e < max(nkt, 4 + nec) and e < 4 else f"mask{e - 4}"
            t = p_per.tile([P, Lc], bf16, tag=tag)
            nc.sync.dma_start(out=t[:], in_=d_ctxT16[e * P:(e + 1) * P, :])
            ctx16.append(t)
        w = load_w(d_w["ca_q"])
        q_raw = project_T(w, h2, Lq, "scrC", p_scr)
        qt2 = []
        for pr in range(npr):
            q = p_per.tile([P, Lq], bf16, tag=f"qt{pr}")
            rope_combine(q[:], q_raw[pr], cq[:], sq[:], Lq)
            qt2.append(q)
        w = load_w(d_w["ca_k"])
        k_raw = project_T(w, ctx16, Lc, "scrA", p_scr)
        kt2 = []
        for e in range(nec):
            full = p_big.tile([P, Lc], bf16, tag="big")
            rope_combine(full[:], k_raw[e], ccos[:], csin[:], Lc)
            kt2.append(full)
        w = load_w(d_w["ca_v"])
        v2 = v_project(w, ctx16, nct, 0)
        on2 = attention(qt2, kt2, v2, nct, False)
        w = load_w(d_w["ca_p"])
        proj_residual(w, on2, 1)

        # ==================== STAGE 3: MLP ==============================
        x16_3 = []
        for e in range(nec):
            t = p_scr.tile([P, Lq], bf16, tag=f"scrB{e}")
            nc.scalar.copy(t[:], xT[e][:])
            x16_3.append(t)
        h3 = layer_norm(x16_3, Lq, "scrA", src_f32=xT)
        a_tiles = []
        for grp in range(NG):
            wf = []
            for e in range(nec):
                t = p_big.tile([P, FG], bf16, tag="big")
                nc.sync.dma_start(
                    out=t[:], in_=d_fc1[e * P:(e + 1) * P,
                                        grp * FG:(grp + 1) * FG])
                wf.append(t)
            for ft in range(nftg):
                ps = ps_mm.tile([P, Lq], f32, tag="mm")
                for ei in range(nec):
                    nc.tensor.matmul(ps[:], wf[ei][:, ft * P:(ft + 1) * P],
                                     h3[ei][:], start=(ei == 0),
                                     stop=(ei == nec - 1))
                a = p_s1k.tile([P, Lq], bf16, tag="s1k")
                nc.scalar.activation(out=a[:], in_=ps[:],
                                     func=AF.Gelu_apprx_tanh)
                a_tiles.append(a)
        # fc2: occupy all 8 psum banks as accumulators, stream fc2 weights
        acc_pools = [ps_mm, ps_mm, ps_av, ps_av, ps_st, ps_st, ps_bc, ps_bc]
        acc_tags = ["mm", "mm", "av", "av", "s1", "s2", "bc", "bc"]
        accs = []
        for e in range(nec):
            pl = acc_pools[e % 8]
            accs.append(pl.tile([P, Lq], f32, tag=acc_tags[e % 8]))
        for fi in range(nft):
            wt = p_w.tile([P, c.E], bf16, tag="wproj")
            nc.sync.dma_start(out=wt[:], in_=d_fc2[fi * P:(fi + 1) * P, :])
            for e in range(nec):
                nc.tensor.matmul(accs[e][:], wt[:, e * P:(e + 1) * P],
                                 a_tiles[fi][:], start=(fi == 0),
                                 stop=(fi == nft - 1))
        for e in range(nec):
            nc.vector.scalar_tensor_tensor(
                out=xT[e][:], in0=accs[e][:], scalar=gsb[:, e, 2:3],
                in1=xT[e][:], op0=ALU.mult, op1=ALU.add)

        # ==================== output ====================
        for e in range(nec):
            nc.sync.dma_start(out=d_out[e * P:(e + 1) * P, :], in_=xT[e][:])

    nc.finalize()
    return nc


# ======================================================================
# Host-side preparation
# ======================================================================
def rope_tables(positions, HD, dtype=np.float32):
    inv_freq = 1.0 / (10000.0 ** (np.arange(0, HD, 2, dtype=np.float64) / HD))
    ang = positions[None, :].astype(np.float64) * inv_freq[:, None]
    cos, sin = np.cos(ang), np.sin(ang)
    c64 = np.concatenate([cos, cos], 0)
    s64 = np.concatenate([-sin, sin], 0)
    return (np.concatenate([c64, c64], 0).astype(dtype),
            np.concatenate([s64, s64], 0).astype(dtype))


def rope_perm(E, HD):
    H = E // HD
    perm = np.zeros(E, dtype=np.int64)
    for h in range(H):
        base = h * HD
        perm[base:base + 32] = base + np.arange(0, HD, 2)
        perm[base + 32:base + HD] = base + np.arange(1, HD, 2)
    return perm


def to_bf(a):
    return np.asarray(a, dtype=np.float32).astype(ml_dtypes.bfloat16)


def host_prep(inputs, cfg: Cfg):
    c = cfg
    E, HD = c.E, c.HD
    perm = rope_perm(E, HD)

    def ln_fold(w, nw, do_perm):
        weff = np.asarray(w, np.float64)
        if nw is not None:
            weff = weff * np.asarray(nw, np.float64)[None, :]
        if do_perm:
            weff = weff[perm, :]
        return weff.T

    x = np.asarray(inputs['x'], np.float32)
    ctxv = np.asarray(inputs['context'], np.float32)
    am = np.asarray(inputs['attn_mask'])
    n1w, n2w, n3w = (np.asarray(inputs[k], np.float32).reshape(-1)
                     for k in ('n1_w', 'n2_w', 'n3_w'))
    for nb in ('n1_b', 'n2_b', 'n3_b', 'sa_qb', 'sa_kb', 'sa_vb', 'sa_pb',
               'ca_qb', 'ca_kb', 'ca_vb', 'ca_pb', 'fc1_b', 'fc2_b'):
        assert not np.any(np.asarray(inputs[nb])), f"nonzero bias {nb}"

    shared = {
        'w_sa_q': to_bf(ln_fold(inputs['sa_qw'], n1w, True)),
        'w_sa_k': to_bf(ln_fold(inputs['sa_kw'], n1w, True)),
        'w_sa_v': to_bf(ln_fold(inputs['sa_vw'], n1w, False)),
        'w_sa_p': to_bf(np.asarray(inputs['sa_pw'], np.float64).T),
        'w_ca_q': to_bf(ln_fold(inputs['ca_qw'], n2w, True)),
        'w_ca_k': to_bf(ln_fold(inputs['ca_kw'], None, True)),
        'w_ca_v': to_bf(np.asarray(inputs['ca_vw'], np.float64).T),
        'w_ca_p': to_bf(np.asarray(inputs['ca_pw'], np.float64).T),
        'w_fc1': to_bf(ln_fold(inputs['fc1_w'], n3w, False)),
        'w_fc2': to_bf(np.asarray(inputs['fc2_w'], np.float64).T),
        'gvec': np.ascontiguousarray(np.stack(
            [np.asarray(inputs['g_msa'], np.float32).reshape(-1),
             np.asarray(inputs['g_ca'], np.float32).reshape(-1),
             np.asarray(inputs['g_mlp'], np.float32).reshape(-1)], 1)),
    }
    mask_T = (~am).astype(np.float32).T       # [k, q] multiplicative
    cc_np, sc_np = rope_tables(np.arange(c.Lc), HD)
    shared['cos_c'] = to_bf(cc_np)
    shared['sin_c'] = to_bf(sc_np)

    in_maps = []
    for core in range(c.n_cores):
        b = core // c.qsplit
        qh = core % c.qsplit
        q_lo = qh * c.Lq
        roll = np.roll(np.arange(c.Lk), -q_lo)
        cq_np, sq_np = rope_tables(np.arange(q_lo, q_lo + c.Lq), HD)
        ck_np, sk_np = rope_tables(roll.astype(np.float64), HD)
        xb_T = x[b].T                          # [E, Lk]
        m = dict(shared)
        m.update({
            'xT': np.ascontiguousarray(xb_T[:, q_lo:q_lo + c.Lq]),
            'xT16': np.ascontiguousarray(to_bf(xb_T[:, roll])),
            'ctxT16': np.ascontiguousarray(to_bf(ctxv[b].T)),
            'mask16': np.ascontiguousarray(
                to_bf(mask_T[roll][:, q_lo:q_lo + c.Lq])),
            'cos_q': to_bf(cq_np), 'sin_q': to_bf(sq_np),
            'cos_k': to_bf(ck_np), 'sin_k': to_bf(sk_np),
        })
        in_maps.append(m)
    return in_maps


def assemble_output(results, cfg: Cfg):
    c = cfg
    out = np.zeros((c.B, c.LQ, c.E), np.float32)
    for core in range(c.n_cores):
        b = core // c.qsplit
        qh = core % c.qsplit
        q_lo = qh * c.Lq
        out[b, q_lo:q_lo + c.Lq, :] = np.asarray(results[core]['outT']).T
    return out


_CFG = Cfg(E=1024, H=16, LQ=1024, LC=512, B=4, FF=4096, n_cores=8)
_CACHE = {}


def kernel(**inputs):
    from concourse.bass_utils import run_bass_kernel_spmd
    cfg = _CFG
    in_maps = host_prep(inputs, cfg)
    if 'nc' not in _CACHE:
        _CACHE['nc'] = build_core_program(cfg)
    res = run_bass_kernel_spmd(_CACHE['nc'], in_maps,
                               core_ids=list(range(cfg.n_cores)))
    return assemble_output(res.results, cfg)


# revision 3
# speedup vs baseline: 1.0132x; 1.0132x over previous
"""Trainium2 Bass kernel for nn_DecoderRoPEBlock (B=4, LQ=1024, LC=512,
E=1024, H=16, FF=4096) running SPMD on 8 NeuronCores.

Sharding: 8 cores = (batch, striped query-tiles); zero collectives.
Stage-1 causal self-attention K/V are recomputed per core from the
original x (causality means the pre-residual x suffices), so each core
produces its 512 output rows independently. Causal striping: each core
owns interleaved 128-token query tiles so score/exp/AV work shrinks by
the causal factor uniformly across cores.
"""
import sys
sys.path.insert(0, '/opt/trn_rl_repo')
from contextlib import ExitStack

import numpy as np
import ml_dtypes

import concourse.bass as bass
import concourse.tile as tile
import concourse.mybir as mybir

f32 = mybir.dt.float32
bf16 = mybir.dt.bfloat16
AF = mybir.ActivationFunctionType
ALU = mybir.AluOpType
EPS = 1e-6
P = 128


class Cfg:
    def __init__(self, E, H, LQ, LC, B, FF, n_cores):
        self.E, self.H, self.LQ, self.LC, self.B, self.FF = E, H, LQ, LC, B, FF
        self.HD = E // H
        assert self.HD == 64, "rope layout assumes head dim 64"
        self.n_cores = n_cores
        self.qsplit = n_cores // B
        assert B * self.qsplit == n_cores
        self.Lq = LQ // self.qsplit
        assert self.Lq <= 512
        self.Lk = LQ
        self.Lc = LC
        self.nec = E // P
        self.nkt = self.Lk // P
        self.nct = self.Lc // P
        self.npr = H // 2
        assert self.npr == self.nec, "2 heads per 128-chunk layout"
        self.nft = FF // P
        self.NT = 512


def _swap32_dma(nc, dst, src, L):
    """dst = src with 32-blocks swapped inside each 64-block (partitions)."""
    ps_d = dst.ap[0][0]
    ps_s = src.ap[0][0]

    def view(ap, ps, poff):
        return bass.AP(tensor=ap.tensor, offset=ap.offset + poff * ps,
                       ap=[[64 * ps, 2], [ps, 32], [1, L]])
    nc.sync.dma_start(out=view(dst, ps_d, 0), in_=view(src, ps_s, 32))
    nc.sync.dma_start(out=view(dst, ps_d, 32), in_=view(src, ps_s, 0))


def build_core_program(cfg: Cfg):
    c = cfg
    nc = bass.Bass()

    d_xT = nc.declare_dram_parameter("xT", [c.E, c.Lq], f32, isOutput=False)
    d_xT16 = nc.declare_dram_parameter("xT16", [c.E, c.Lk], bf16, isOutput=False)
    d_ctxT16 = nc.declare_dram_parameter("ctxT16", [c.E, c.Lc], bf16, isOutput=False)
    d_mask = nc.declare_dram_parameter("mask16", [c.Lk, c.Lq], bf16, isOutput=False)
    WNAMES = ["sa_q", "sa_k", "sa_v", "sa_p", "ca_q", "ca_k", "ca_v", "ca_p"]
    d_w = {n: nc.declare_dram_parameter("w_" + n, [c.E, c.E], bf16, isOutput=False)
           for n in WNAMES}
    d_fc1 = nc.declare_dram_parameter("w_fc1", [c.E, c.FF], bf16, isOutput=False)
    d_fc2 = nc.declare_dram_parameter("w_fc2", [c.FF, c.E], bf16, isOutput=False)
    d_cq = nc.declare_dram_parameter("cos_q", [P, c.Lq], bf16, isOutput=False)
    d_sq = nc.declare_dram_parameter("sin_q", [P, c.Lq], bf16, isOutput=False)
    d_ck = nc.declare_dram_parameter("cos_k", [P, c.Lk], bf16, isOutput=False)
    d_sk = nc.declare_dram_parameter("sin_k", [P, c.Lk], bf16, isOutput=False)
    d_cc = nc.declare_dram_parameter("cos_c", [P, c.Lc], bf16, isOutput=False)
    d_sc = nc.declare_dram_parameter("sin_c", [P, c.Lc], bf16, isOutput=False)
    d_g = nc.declare_dram_parameter("gvec", [c.E, 3], f32, isOutput=False)
    d_out = nc.declare_dram_parameter("outT", [c.E, c.Lq], f32, isOutput=True)

    Lq, Lk, Lc, nec, nkt, nct, npr, nft = (
        c.Lq, c.Lk, c.Lc, c.nec, c.nkt, c.nct, c.npr, c.nft)
    VNT = min(c.NT, c.E)
    n_vnt = c.E // VNT
    KNT = min(c.NT, Lk)
    n_knt = Lk // KNT
    NG = 4 if nft % 4 == 0 else 1          # fc1/fc2 stream groups
    FG = c.FF // NG                        # cols per fc1 group
    nftg = nft // NG

    with ExitStack() as ctx, tile.TileContext(nc) as tc:
        # -------------------- pools --------------------
        p_x = ctx.enter_context(tc.tile_pool(name="p_x", bufs=1))
        p_h = ctx.enter_context(tc.tile_pool(name="p_h", bufs=1))
        p_big = ctx.enter_context(tc.tile_pool(name="p_big", bufs=9))
        p_s1k = ctx.enter_context(tc.tile_pool(name="p_s1k", bufs=34))
        p_w = ctx.enter_context(tc.tile_pool(name="p_w", bufs=10))
        p_per = ctx.enter_context(tc.tile_pool(name="p_per", bufs=1))
        p_scr = ctx.enter_context(tc.tile_pool(name="p_scr", bufs=1))
        p_sm = ctx.enter_context(tc.tile_pool(name="p_sm", bufs=4))
        p_t = ctx.enter_context(tc.tile_pool(name="p_t", bufs=4))
        ps_mm = ctx.enter_context(tc.tile_pool(name="ps_mm", bufs=2, space="PSUM"))
        ps_av = ctx.enter_context(tc.tile_pool(name="ps_av", bufs=2, space="PSUM"))
        ps_st = ctx.enter_context(tc.tile_pool(name="ps_st", bufs=1, space="PSUM"))
        ps_bc = ctx.enter_context(tc.tile_pool(name="ps_bc", bufs=2, space="PSUM"))

        # -------------------- prologue loads --------------------
        xT = []
        for e in range(nec):
            t = p_x.tile([P, Lq], f32, tag=f"x{e}")
            nc.sync.dma_start(out=t[:], in_=d_xT[e * P:(e + 1) * P, :])
            xT.append(t)
        x16 = []
        for e in range(nec):
            t = p_h.tile([P, Lk], bf16, tag=f"h{e}")
            nc.sync.dma_start(out=t[:], in_=d_xT16[e * P:(e + 1) * P, :])
            x16.append(t)
        masks = []
        for kt in range(nkt):
            t = p_per.tile([P, Lq], bf16, tag=f"mask{kt}")
            nc.sync.dma_start(out=t[:], in_=d_mask[kt * P:(kt + 1) * P, :])
            masks.append(t)
        cq = p_per.tile([P, Lq], bf16, tag="cq")
        sq = p_per.tile([P, Lq], bf16, tag="sq")
        ck = p_per.tile([P, Lk], bf16, tag="ck")
        sk = p_per.tile([P, Lk], bf16, tag="sk")
        ccos = p_per.tile([P, Lc], bf16, tag="ccos")
        csin = p_per.tile([P, Lc], bf16, tag="csin")
        for t, d in ((cq, d_cq), (sq, d_sq), (ck, d_ck), (sk, d_sk),
                     (ccos, d_cc), (csin, d_sc)):
            nc.sync.dma_start(out=t[:], in_=d[:, :])
        gsb = p_per.tile([P, nec, 3], f32, tag="g")
        for e in range(nec):
            nc.sync.dma_start(out=gsb[:, e, :], in_=d_g[e * P:(e + 1) * P, :])
        ones_col = p_per.tile([P, 1], bf16, tag="ones_col")
        nc.vector.memset(ones_col[:], 1.0)
        ones_row = p_per.tile([1, P], bf16, tag="ones_row")
        nc.vector.memset(ones_row[:], 1.0)
        epsb = p_per.tile([1, 1], f32, tag="epsb")
        nc.vector.memset(epsb[:], EPS)

        def load_w(dram, tag="wproj"):
            tiles = []
            for e in range(nec):
                t = p_w.tile([P, c.E], bf16, tag=tag)
                nc.sync.dma_start(out=t[:], in_=dram[e * P:(e + 1) * P, :])
                tiles.append(t)
            return tiles

        # ==================== LN ====================
        def layer_norm(src_tiles, L, sq_tag, src_f32=None):
            """LN over E of transposed src [E, L] -> bf16 h tiles (p_h h{e})."""
            n_lt = max(1, L // 512)
            LT = L // n_lt
            sq_t = []
            for e in range(nec):
                s = p_scr.tile([P, L], bf16, tag=f"{sq_tag}{e}")
                nc.vector.tensor_mul(s[:], src_tiles[e][:], src_tiles[e][:])
                sq_t.append(s)
            hs = [p_h.tile([P, L], bf16, tag=f"h{e}") for e in range(nec)]
            for lt in range(n_lt):
                sl = slice(lt * LT, (lt + 1) * LT)
                s1 = ps_st.tile([1, LT], f32, tag="s1")
                s2 = ps_st.tile([1, LT], f32, tag="s2")
                for e in range(nec):
                    nc.tensor.matmul(s1[:], ones_col[:], src_tiles[e][:, sl],
                                     start=(e == 0), stop=(e == nec - 1))
                for e in range(nec):
                    nc.tensor.matmul(s2[:], ones_col[:], sq_t[e][:, sl],
                                     start=(e == 0), stop=(e == nec - 1))
                mu = p_sm.tile([1, LT], f32, tag="mu")
                nc.scalar.mul(mu[:], s1[:], 1.0 / c.E)
                mu2 = p_sm.tile([1, LT], f32, tag="mu2")
                nc.scalar.square(mu2[:], mu[:])
                var = p_sm.tile([1, LT], f32, tag="var")
                nc.vector.scalar_tensor_tensor(
                    out=var[:], in0=s2[:], scalar=1.0 / c.E, in1=mu2[:],
                    op0=ALU.mult, op1=ALU.subtract)
                lnv = p_sm.tile([1, LT], f32, tag="lnv")
                nc.scalar.activation(out=lnv[:], in_=var[:], func=AF.Ln,
                                     bias=epsb[:])
                rstd = p_sm.tile([1, LT], bf16, tag="rstd")
                nc.scalar.activation(out=rstd[:], in_=lnv[:], func=AF.Exp,
                                     scale=-0.5)
                ccv = p_sm.tile([1, LT], bf16, tag="ccv")
                nc.vector.tensor_mul(ccv[:], mu[:], rstd[:])
                rstd_b = ps_bc.tile([P, LT], f32, tag="bc")
                nc.tensor.matmul(rstd_b[:], ones_row[:], rstd[:],
                                 start=True, stop=True)
                cc_b = ps_bc.tile([P, LT], f32, tag="bc")
                nc.tensor.matmul(cc_b[:], ones_row[:], ccv[:],
                                 start=True, stop=True)
                for e in range(nec):
                    src = src_f32[e] if src_f32 is not None else src_tiles[e]
                    tmp = p_t.tile([P, LT], f32, tag="lntmp")
                    nc.vector.tensor_mul(tmp[:], src[:, sl], rstd_b[:])
                    nc.vector.tensor_sub(hs[e][:, sl], tmp[:], cc_b[:])
            return hs

        # ==================== projections ====================
        def project_T(w_tiles, rhs_tiles, L, out_tag, pool):
            """[E_out, L] = sum_e w[e].T @ rhs[e]; returns nec bf16 tiles."""
            outs = []
            for eo in range(nec):
                ps = ps_mm.tile([P, L], f32, tag="mm")
                for ei in range(nec):
                    nc.tensor.matmul(ps[:], w_tiles[ei][:, eo * P:(eo + 1) * P],
                                     rhs_tiles[ei][:], start=(ei == 0),
                                     stop=(ei == nec - 1))
                o = pool.tile([P, L], bf16, tag=f"{out_tag}{eo}")
                nc.scalar.copy(o[:], ps[:])
                outs.append(o)
            return outs

        def rope_combine(dst_ap, raw_tile, cos_ap, sin_ap, L):
            swp = p_t.tile([P, L], bf16, tag="ropeswp")
            _swap32_dma(nc, swp[:], raw_tile[:], L)
            t1 = p_t.tile([P, L], bf16, tag="ropet1")
            nc.vector.tensor_mul(t1[:], raw_tile[:], cos_ap)
            t2 = p_t.tile([P, L], bf16, tag="ropet2")
            nc.vector.tensor_mul(t2[:], swp[:], sin_ap)
            nc.vector.tensor_add(dst_ap, t1[:], t2[:])

        def v_project(w_tiles, rhs_tiles, n_kt, rhs_off):
            """V in token layout with per-head ones column: [k, H*65]."""
            v_sb = []
            for kt in range(n_kt):
                vt = p_per.tile([P, c.H, 65], bf16, tag=f"v{kt}")
                nc.vector.memset(vt[:, :, 64:65], 1.0)
                for vn in range(n_vnt):
                    ps = ps_mm.tile([P, VNT], f32, tag="mm")
                    for ei in range(nec):
                        nc.tensor.matmul(
                            ps[:],
                            rhs_tiles[ei][:, rhs_off + kt * P:rhs_off + (kt + 1) * P],
                            w_tiles[ei][:, vn * VNT:(vn + 1) * VNT],
                            start=(ei == 0), stop=(ei == nec - 1))
                    nh = VNT // 64
                    nc.scalar.copy(
                        vt[:, vn * nh:(vn + 1) * nh, 0:64],
                        ps[:].rearrange("p (nh d) -> p nh d", d=64))
                v_sb.append(vt)
            return v_sb

        # ==================== attention ====================
        def attention(q_tiles, k_tiles, v_sb, n_kt, use_mask):
            """Returns Onorm tiles (aliasing q_tiles' slots, tag qt{pr})."""
            on_tiles = []
            for pr in range(npr):
                qch = q_tiles[pr]
                kch = k_tiles[pr]
                results = []  # (pbase, o_ps, db)
                for hh, pbase in ((2 * pr, 0), (2 * pr + 1, 64)):
                    o_ps = ps_av.tile([65, Lq], f32, tag="av")
                    exps = []
                    for kt in range(n_kt):
                        s_ps = ps_mm.tile([P, Lq], f32, tag="mm")
                        nc.tensor.matmul(
                            s_ps[:],
                            kch[pbase:pbase + 64, kt * P:(kt + 1) * P],
                            qch[pbase:pbase + 64, :],
                            start=True, stop=True)
                        ex = p_s1k.tile([P, Lq], bf16, tag="s1k")
                        nc.scalar.activation(out=ex[:], in_=s_ps[:],
                                             func=AF.Exp, scale=0.125)
                        if use_mask:
                            exm = p_s1k.tile([P, Lq], bf16, tag="s1k")
                            nc.vector.tensor_mul(exm[:], ex[:], masks[kt][:])
                            ex = exm
                        exps.append(ex)
                    for kt in range(n_kt):
                        nc.tensor.matmul(o_ps[:],
                                         v_sb[kt][:, hh, :],
                                         exps[kt][:],
                                         start=(kt == 0), stop=(kt == n_kt - 1))
                    rec = p_sm.tile([1, Lq], bf16, tag="rec")
                    nc.vector.reciprocal(rec[:], o_ps[64:65, :])
                    db_ps = ps_bc.tile([P, Lq], f32, tag="bc")
                    nc.tensor.matmul(db_ps[0:64, :], ones_row[:, 0:64], rec[:],
                                     start=True, stop=True)
                    db = p_t.tile([64, Lq], bf16, tag="db")
                    nc.scalar.copy(db[:], db_ps[0:64, :])
                    results.append((pbase, o_ps, db))
                # Onorm tile reuses the pair's q slot (q dead after scores)
                on = p_per.tile([P, Lq], bf16, tag=f"qt{pr}")
                for pbase, o_ps, db in results:
                    nc.vector.scalar_tensor_tensor(
                        out=on[pbase:pbase + 64, :],
                        in0=o_ps[0:64, :], scalar=1.0, in1=db[:],
                        op0=ALU.bypass, op1=ALU.mult)
                on_tiles.append(on)
            return on_tiles

        def proj_residual(w_tiles, src_tiles, g_idx):
            for e in range(nec):
                ps = ps_mm.tile([P, Lq], f32, tag="mm")
                for ei in range(nec):
                    nc.tensor.matmul(ps[:], w_tiles[ei][:, e * P:(e + 1) * P],
                                     src_tiles[ei][:], start=(ei == 0),
                                     stop=(ei == nec - 1))
                nc.vector.scalar_tensor_tensor(
                    out=xT[e][:], in0=ps[:], scalar=gsb[:, e, g_idx:g_idx + 1],
                    in1=xT[e][:], op0=ALU.mult, op1=ALU.add)

        # ==================== STAGE 1: causal self-attention ============
        h1 = layer_norm(x16, Lk, "scrA")
        hq = [t[:, 0:Lq] for t in h1]

        w = load_w(d_w["sa_q"])
        q_raw = project_T(w, hq, Lq, "scrC", p_scr)
        qt1 = []
        for pr in range(npr):
            q = p_per.tile([P, Lq], bf16, tag=f"qt{pr}")
            rope_combine(q[:], q_raw[pr], cq[:], sq[:], Lq)
            qt1.append(q)
        w = load_w(d_w["sa_k"])
        kt1 = []
        k_raw_nt = []
        for nt in range(n_knt):
            sl = slice(nt * KNT, (nt + 1) * KNT)
            tag = "scrA" if nt == 0 else "scrB"
            kr = project_T(w, [t[:, sl] for t in h1], KNT, tag, p_scr)
            k_raw_nt.append(kr)
        for e in range(nec):
            full = p_big.tile([P, Lk], bf16, tag="big")
            for nt in range(n_knt):
                sl = slice(nt * KNT, (nt + 1) * KNT)
                rope_combine(full[:, sl], k_raw_nt[nt][e],
                             ck[:, sl], sk[:, sl], KNT)
            kt1.append(full)
        w = load_w(d_w["sa_v"])
        v1 = v_project(w, h1, nkt, 0)
        on1 = attention(qt1, kt1, v1, nkt, True)
        w = load_w(d_w["sa_p"])
        proj_residual(w, on1, 0)

        # ==================== STAGE 2: cross-attention ==================
        x16_2 = []
        for e in range(nec):
            t = p_scr.tile([P, Lq], bf16, tag=f"scrB{e}")
            nc.scalar.copy(t[:], xT[e][:])
            x16_2.append(t)
        h2 = layer_norm(x16_2, Lq, "scrA", src_f32=xT)
        ctx16 = []
        for e in range(nec):
            tag = f"v{4 + e}" if 4 + e < max(nkt, 4 + nec) and e < 4 else f"mask{e - 4}"
            t = p_per.tile([P, Lc], bf16, tag=tag)
            nc.sync.dma_start(out=t[:], in_=d_ctxT16[e * P:(e + 1) * P, :])
            ctx16.append(t)
        w = load_w(d_w["ca_q"])
        q_raw = project_T(w, h2, Lq, "scrC", p_scr)
        qt2 = []
        for pr in range(npr):
            q = p_per.tile([P, Lq], bf16, tag=f"qt{pr}")
            rope_combine(q[:], q_raw[pr], cq[:], sq[:], Lq)
            qt2.append(q)
        w = load_w(d_w["ca_k"])
        k_raw = project_T(w, ctx16, Lc, "scrA", p_scr)
        kt2 = []
        for e in range(nec):
            full = p_big.tile([P, Lc], bf16, tag="big")
            rope_combine(full[:], k_raw[e], ccos[:], csin[:], Lc)
            kt2.append(full)
        w = load_w(d_w["ca_v"])
        v2 = v_project(w, ctx16, nct, 0)
        on2 = attention(qt2, kt2, v2, nct, False)
        w = load_w(d_w["ca_p"])
        proj_residual(w, on2, 1)

        # ==================== STAGE 3: MLP ==============================
        x16_3 = []
        for e in range(nec):
            t = p_scr.tile([P, Lq], bf16, tag=f"scrB{e}")
            nc.scalar.copy(t[:], xT[e][:])
            x16_3.append(t)
        h3 = layer_norm(x16_3, Lq, "scrA", src_f32=xT)
        a_tiles = []
        for grp in range(NG):
            wf = []
            for e in range(nec):
                t = p_big.tile([P, FG], bf16, tag="big")
                nc.sync.dma_start(
                    out=t[:], in_=d_fc1[e * P:(e + 1) * P,
                                        grp * FG:(grp + 1) * FG])
                wf.append(t)
            for ft in range(nftg):
                ps = ps_mm.tile([P, Lq], f32, tag="mm")
                for ei in range(nec):
                    nc.tensor.matmul(ps[:], wf[ei][:, ft * P:(ft + 1) * P],
                                     h3[ei][:], start=(ei == 0),
                                     stop=(ei == nec - 1))
                a = p_s1k.tile([P, Lq], bf16, tag="s1k")
                nc.scalar.activation(out=a[:], in_=ps[:],
                                     func=AF.Gelu_apprx_tanh)
                a_tiles.append(a)
        # fc2: occupy all 8 psum banks as accumulators, stream fc2 weights
        acc_pools = [ps_mm, ps_mm, ps_av, ps_av, ps_st, ps_st, ps_bc, ps_bc]
        acc_tags = ["mm", "mm", "av", "av", "s1", "s2", "bc", "bc"]
        accs = []
        for e in range(nec):
            pl = acc_pools[e % 8]
            accs.append(pl.tile([P, Lq], f32, tag=acc_tags[e % 8]))
        for fi in range(nft):
            wt = p_w.tile([P, c.E], bf16, tag="wproj")
            nc.sync.dma_start(out=wt[:], in_=d_fc2[fi * P:(fi + 1) * P, :])
            for e in range(nec):
                nc.tensor.matmul(accs[e][:], wt[:, e * P:(e + 1) * P],
                                 a_tiles[fi][:], start=(fi == 0),
                                 stop=(fi == nft - 1))
        for e in range(nec):
            nc.vector.scalar_tensor_tensor(
                out=xT[e][:], in0=accs[e][:], scalar=gsb[:, e, 2:3],
                in1=xT[e][:], op0=ALU.mult, op1=ALU.add)

        # ==================== output ====================
        for e in range(nec):
            nc.sync.dma_start(out=d_out[e * P:(e + 1) * P, :], in_=xT[e][:])

    nc.finalize()
    return nc


# ======================================================================
# Host-side preparation
# ======================================================================
def rope_tables(positions, HD, dtype=np.float32):
    inv_freq = 1.0 / (10000.0 ** (np.arange(0, HD, 2, dtype=np.float64) / HD))
    ang = positions[None, :].astype(np.float64) * inv_freq[:, None]
    cos, sin = np.cos(ang), np.sin(ang)
    c64 = np.concatenate([cos, cos], 0)
    s64 = np.concatenate([-sin, sin], 0)
    return (np.concatenate([c64, c64], 0).astype(dtype),
            np.concatenate([s64, s64], 0).astype(dtype))


def rope_perm(E, HD):
    H = E // HD
    perm = np.zeros(E, dtype=np.int64)
    for h in range(H):
        base = h * HD
        perm[base:base + 32] = base + np.arange(0, HD, 2)
        perm[base + 32:base + HD] = base + np.arange(1, HD, 2)
    return perm


def to_bf(a):
    return np.asarray(a, dtype=np.float32).astype(ml_dtypes.bfloat16)


def host_prep(inputs, cfg: Cfg):
    c = cfg
    E, HD = c.E, c.HD
    perm = rope_perm(E, HD)

    def ln_fold(w, nw, do_perm):
        weff = np.asarray(w, np.float64)
        if nw is not None:
            weff = weff * np.asarray(nw, np.float64)[None, :]
        if do_perm:
            weff = weff[perm, :]
        return weff.T

    x = np.asarray(inputs['x'], np.float32)
    ctxv = np.asarray(inputs['context'], np.float32)
    am = np.asarray(inputs['attn_mask'])
    n1w, n2w, n3w = (np.asarray(inputs[k], np.float32).reshape(-1)
                     for k in ('n1_w', 'n2_w', 'n3_w'))
    for nb in ('n1_b', 'n2_b', 'n3_b', 'sa_qb', 'sa_kb', 'sa_vb', 'sa_pb',
               'ca_qb', 'ca_kb', 'ca_vb', 'ca_pb', 'fc1_b', 'fc2_b'):
        assert not np.any(np.asarray(inputs[nb])), f"nonzero bias {nb}"

    shared = {
        'w_sa_q': to_bf(ln_fold(inputs['sa_qw'], n1w, True)),
        'w_sa_k': to_bf(ln_fold(inputs['sa_kw'], n1w, True)),
        'w_sa_v': to_bf(ln_fold(inputs['sa_vw'], n1w, False)),
        'w_sa_p': to_bf(np.asarray(inputs['sa_pw'], np.float64).T),
        'w_ca_q': to_bf(ln_fold(inputs['ca_qw'], n2w, True)),
        'w_ca_k': to_bf(ln_fold(inputs['ca_kw'], None, True)),
        'w_ca_v': to_bf(np.asarray(inputs['ca_vw'], np.float64).T),
        'w_ca_p': to_bf(np.asarray(inputs['ca_pw'], np.float64).T),
        'w_fc1': to_bf(ln_fold(inputs['fc1_w'], n3w, False)),
        'w_fc2': to_bf(np.asarray(inputs['fc2_w'], np.float64).T),
        'gvec': np.ascontiguousarray(np.stack(
            [np.asarray(inputs['g_msa'], np.float32).reshape(-1),
             np.asarray(inputs['g_ca'], np.float32).reshape(-1),
             np.asarray(inputs['g_mlp'], np.float32).reshape(-1)], 1)),
    }
    mask_T = (~am).astype(np.float32).T       # [k, q] multiplicative
    cc_np, sc_np = rope_tables(np.arange(c.Lc), HD)
    shared['cos_c'] = to_bf(cc_np)
    shared['sin_c'] = to_bf(sc_np)

    in_maps = []
    for core in range(c.n_cores):
        b = core // c.qsplit
        qh = core % c.qsplit
        q_lo = qh * c.Lq
        roll = np.roll(np.arange(c.Lk), -q_lo)
        cq_np, sq_np = rope_tables(np.arange(q_lo, q_lo + c.Lq), HD)
        ck_np, sk_np = rope_tables(roll.astype(np.float64), HD)
        xb_T = x[b].T                          # [E, Lk]
        m = dict(shared)
        m.update({
            'xT': np.ascontiguousarray(xb_T[:, q_lo:q_lo + c.Lq]),
            'xT16': np.ascontiguousarray(to_bf(xb_T[:, roll])),
            'ctxT16': np.ascontiguousarray(to_bf(ctxv[b].T)),
            'mask16': np.ascontiguousarray(
                to_bf(mask_T[roll][:, q_lo:q_lo + c.Lq])),
            'cos_q': to_bf(cq_np), 'sin_q': to_bf(sq_np),
            'cos_k': to_bf(ck_np), 'sin_k': to_bf(sk_np),
        })
        in_maps.append(m)
    return in_maps


def assemble_output(results, cfg: Cfg):
    c = cfg
    out = np.zeros((c.B, c.LQ, c.E), np.float32)
    for core in range(c.n_cores):
        b = core // c.qsplit
        qh = core % c.qsplit
        q_lo = qh * c.Lq
        out[b, q_lo:q_lo + c.Lq, :] = np.asarray(results[core]['outT']).T
    return out


_CFG = Cfg(E=1024, H=16, LQ=1024, LC=512, B=4, FF=4096, n_cores=8)
_CACHE = {}


def kernel(**inputs):
    from concourse.bass_utils import run_bass_kernel_spmd
    cfg = _CFG
    in_maps = host_prep(inputs, cfg)
    if 'nc' not in _CACHE:
        _CACHE['nc'] = build_core_program(cfg)
    res = run_bass_kernel_spmd(_CACHE['nc'], in_maps,
                               core_ids=list(range(cfg.n_cores)))
    return assemble_output(res.results, cfg)
